# revision 10
# baseline (speedup 1.0000x reference)
"""MoE routing kernel for Trainium2, 8 NeuronCores, expert parallelism.

Strategy (per core c, one expert e=c per core, SPMD uniform program):
  1. Gating on the core's token shard (1/8 of tokens): scores = softmax(x @ gate_w.T),
     top-2 selection via max8, masked weights w_e = prob_e * (logit_e >= 2nd max).
  2. AllGather the per-shard masked-weight table [NSH+1, E] (row NSH carries the
     shard's softmax-prob column sums for the aux loss).
  3. Every core rebuilds full routing locally: per-expert inclusive scan along
     tokens (tensor_tensor_scan + strictly-upper-triangular matmul for the
     cross-partition exclusive prefix) -> global capacity ranks in token order,
     exactly matching the reference's cumsum-based dispatch.
  4. Dispatch for own expert: slot compaction via equality-matmul
     (idx[s] = sum_t [off_t == s] * t), indirect-DMA row gather of x,
     PE transposes, then the expert FFN  y = silu(x_e @ w1.T) @ w2.T
     in fp32r (full PE rate at free dim >= 256).
  5. AllGather the compact per-expert outputs [SLOTS, D]; each core combines its
     own tokens: out[t] = w1*y_all[slot1] + w2*y_all[slot2] via indirect gathers.

kernel(x, gate_w, w1, w2) takes full inputs, returns (out[B,T,D], aux_loss).
"""

import sys

sys.path.insert(0, "/opt/trn_rl_repo")

from contextlib import ExitStack
from dataclasses import dataclass, field

import numpy as np

import concourse.bass as bass
import concourse.mybir as mybir
import concourse.tile as tile
from concourse import bacc
from concourse.bass_utils import run_bass_kernel_spmd
from concourse.masks import make_identity, make_upper_triangular

F32 = mybir.dt.float32
F32R = mybir.dt.float32r
I32 = mybir.dt.int32
AF = mybir.ActivationFunctionType
OP = mybir.AluOpType

N_CORES = 8
P = 128


@dataclass
class Cfg:
    B: int = 4
    T: int = 2048
    D: int = 1024
    H: int = 4096
    E: int = 8
    top_k: int = 2
    cap_factor: float = 1.25
    aux_coef: float = 0.01
    ffn_dtype: mybir.dt = F32R

    N: int = field(init=False)
    NSH: int = field(init=False)   # tokens per shard (per core)
    BLK: int = field(init=False)   # tokens per partition row (t = p*BLK + b)
    cap: int = field(init=False)
    SLOTS: int = field(init=False)  # padded slot count (multiple of 128, > cap)

    def __post_init__(self):
        self.N = self.B * self.T
        self.NSH = self.N // N_CORES
        self.BLK = self.N // P
        assert self.NSH % P == 0 and self.N % P == 0
        assert self.BLK <= 64, "b index must stay fp32r-exact and <= 64"
        self.cap = int(self.cap_factor * self.N / self.E) + 1
        self.SLOTS = ((self.cap + 1 + P - 1) // P) * P

    # ---- tiling helpers ----
    def ffn_chunks(self):
        """Slot ranges for the FFN, 128-aligned, free-dim bounded for SBUF."""
        out, s = [], 0
        # first chunk smaller to start compute earlier; all chunks <= 768 wide
        sizes = []
        rem = self.SLOTS
        first = min(640, rem)
        sizes.append(first)
        rem -= first
        while rem > 0:
            w = min(768, rem)
            sizes.append(w)
            rem -= w
        for w in sizes:
            out.append((s, w))
            s += w
        return out

    def mm_subs(self, w):
        """Split width w into moving-operand pieces <= 512 (>=256 when possible)."""
        subs, s = [], 0
        while w - s > 512:
            piece = 384 if (w - s - 384) >= 256 else 256
            subs.append((s, piece))
            s += piece
        subs.append((s, w - s))
        return subs

    def eq_chunks(self):
        """Slot ranges for compaction equality sweep over [0, ~cap).

        Widths must be even (fp32r matmul moving dim), so the sweep covers
        [0, cap rounded up to even); the extra slot never matches any offset
        (offsets are either < cap or BIG) and is harmless.
        """
        cap_e = self.cap + (self.cap % 2)
        out, s = [], 0
        while s < cap_e:
            w = min(512, cap_e - s)
            out.append((s, w))
            s += w
        return out


def ceil_div(a, b):
    return (a + b - 1) // b


def build_nc(cfg: Cfg):
    nc = bacc.Bacc("TRN2", target_bir_lowering=False, debug=False,
                   num_devices=N_CORES)
    N, D, H, E = cfg.N, cfg.D, cfg.H, cfg.E
    NSH, BLK, cap, SLOTS = cfg.NSH, cfg.BLK, cfg.cap, cfg.SLOTS
    ND = D // P    # d chunks
    NH = H // P    # h chunks
    NRT = NSH // P  # token tiles per shard
    SPP = NSH // BLK  # partition rows per shard (=16)
    NG = SLOTS // P  # slot groups of 128
    FD = cfg.ffn_dtype
    BIG = float(2 * SLOTS + 37)

    # ---------------- I/O ----------------
    x_full = nc.dram_tensor("x_full", [N, D], F32, kind="ExternalInput")
    xTs = nc.dram_tensor("xTs", [D, NSH], F32, kind="ExternalInput")
    gwT = nc.dram_tensor("gwT", [D, E], F32, kind="ExternalInput")
    w1t = nc.dram_tensor("w1t", [D, H], F32, kind="ExternalInput")
    w2t = nc.dram_tensor("w2t", [H, D], F32, kind="ExternalInput")
    esel = nc.dram_tensor("esel", [P, E], F32, kind="ExternalInput")
    tok_ids = nc.dram_tensor("tok_ids", [NSH, 1], I32, kind="ExternalInput")

    out_shard = nc.dram_tensor("out_shard", [NSH, D], F32, kind="ExternalOutput")
    aux_out = nc.dram_tensor("aux_out", [1, 1], F32, kind="ExternalOutput")

    pay_loc = nc.dram_tensor("pay_loc", [NSH + 1, E], F32)
    pay_all = nc.dram_tensor("pay_all", [(NSH + 1) * N_CORES, E], F32,
                             addr_space="Shared")
    meta = nc.dram_tensor("meta", [N, 4], F32)
    y_loc = nc.dram_tensor("y_loc", [SLOTS, D], F32)
    y_all = nc.dram_tensor("y_all", [SLOTS * N_CORES, D], F32,
                           addr_space="Shared")

    rg = [list(range(N_CORES))]

    with tile.TileContext(nc) as tc, ExitStack() as top:
        const = top.enter_context(tc.tile_pool(name="const", bufs=1))

        ident = const.tile([P, P], F32, tag="ident")
        make_identity(nc, ident[:])
        sut = const.tile([P, P], F32, tag="sut")
        make_upper_triangular(nc, sut[:], val=1.0, diag=False)  # 1 iff row < col

        iota_p_i = const.tile([P, 1], I32, tag="iotapi")
        nc.gpsimd.iota(iota_p_i[:], pattern=[[0, 1]], base=0, channel_multiplier=1)
        iota_p = const.tile([P, 1], F32, tag="iotap")
        nc.vector.tensor_copy(iota_p[:], iota_p_i[:])

        iota_b_i = const.tile([P, BLK], I32, tag="iotabi")
        nc.gpsimd.iota(iota_b_i[:], pattern=[[1, BLK]], base=0, channel_multiplier=0)
        iota_b = const.tile([P, BLK], F32, tag="iotab")
        nc.vector.tensor_copy(iota_b[:], iota_b_i[:])

        ones = const.tile([P, 1], F32, tag="ones")
        nc.vector.memset(ones[:], 1.0)
        zerosB = const.tile([P, BLK], F32, tag="zerosB")
        nc.vector.memset(zerosB[:], 0.0)

        eq_iotas = {}
        for (s0, w) in cfg.eq_chunks():
            ii = const.tile([P, w], I32, tag=f"eqii{s0}")
            nc.gpsimd.iota(ii[:], pattern=[[1, w]], base=s0, channel_multiplier=0)
            ff = const.tile([P, w], F32, tag=f"eqif{s0}")
            nc.vector.tensor_copy(ff[:], ii[:])
            eq_iotas[s0] = ff

        esel_sb = const.tile([P, E], F32, tag="esel")
        nc.sync.dma_start(out=esel_sb[:], in_=esel[:])

        gwT_sb = []
        for dc in range(ND):
            g = const.tile([P, E], F32, tag=f"gwT{dc}")
            nc.sync.dma_start(out=g[:], in_=gwT[dc * P:(dc + 1) * P, :])
            gwT_sb.append(g)

        # =========== Phase A: gating on own shard ===========
        with ExitStack() as ph:
            gp = ph.enter_context(tc.tile_pool(name="gate", bufs=1))
            gps = ph.enter_context(tc.tile_pool(name="gates", bufs=3))
            psg = ph.enter_context(tc.tile_pool(name="psg", bufs=2, space="PSUM"))
            psd = ph.enter_context(tc.tile_pool(name="psd", bufs=2, space="PSUM"))

            xTs_sb = []
            for dc in range(ND):
                t = gp.tile([P, NSH], F32, tag=f"xts{dc}")
                nc.sync.dma_start(out=t[:], in_=xTs[dc * P:(dc + 1) * P, :])
                xTs_sb.append(t)

            # scoresT [E, NSH] in <=512 moving pieces (plain fp32 for exactness)
            scoresT = gp.tile([E, NSH], F32, tag="scoresT")
            for (c0, w) in cfg.mm_subs(NSH):
                ps = psg.tile([E, w], F32, tag="gps")
                for dc in range(ND):
                    nc.tensor.matmul(ps[:], lhsT=gwT_sb[dc][:],
                                     rhs=xTs_sb[dc][:, c0:c0 + w],
                                     start=(dc == 0), stop=(dc == ND - 1))
                nc.vector.tensor_copy(scoresT[:, c0:c0 + w], ps[:])

            ss_ps = psd.tile([E, 1], F32, tag="ssps", bufs=1)
            for rt in range(NRT):
                # transpose scores tile -> [128 tokens, E]
                ltp = psd.tile([P, E], F32, tag="ltp")
                nc.tensor.transpose(ltp[:], scoresT[:, rt * P:(rt + 1) * P],
                                    ident[0:E, 0:E])
                logits = gps.tile([P, E], F32, tag="logits")
                nc.vector.tensor_copy(logits[:], ltp[:])

                expv = gps.tile([P, E], F32, tag="expv")
                sume = gps.tile([P, 1], F32, tag="sume")
                nc.scalar.activation(expv[:], logits[:], AF.Exp, accum_out=sume[:])
                rsum = gps.tile([P, 1], F32, tag="rsum")
                nc.vector.reciprocal(rsum[:], sume[:])
                probs = gps.tile([P, E], F32, tag="probs")
                nc.vector.tensor_scalar_mul(probs[:], expv[:], rsum[:])

                m8 = gps.tile([P, 8], F32, tag="m8")
                nc.vector.max(out=m8[:], in_=logits[:])
                chos = gps.tile([P, E], F32, tag="chos")
                nc.vector.tensor_scalar(chos[:], logits[:], m8[:, cfg.top_k - 1:cfg.top_k],
                                        None, op0=OP.is_ge)
                wmask = gps.tile([P, E], F32, tag="wmask")
                nc.vector.tensor_tensor(out=wmask[:], in0=probs[:], in1=chos[:],
                                        op=OP.mult)
                nc.sync.dma_start(out=pay_loc[rt * P:(rt + 1) * P, :], in_=wmask[:])

                nc.tensor.matmul(ss_ps[:], lhsT=probs[:], rhs=ones[:],
                                 start=(rt == 0), stop=(rt == NRT - 1))

            ss_sb = gps.tile([E, 1], F32, tag="sssb")
            nc.vector.tensor_copy(ss_sb[:], ss_ps[:])
            nc.sync.dma_start(out=pay_loc[NSH:NSH + 1, :], in_=ss_sb[:])

            nc.gpsimd.collective_compute(
                "AllGather", OP.bypass, replica_groups=rg,
                ins=[pay_loc[:]], outs=[pay_all[:]],
            )

        # =========== Phase B: routing rebuild (all tokens) ===========
        mid = top.enter_context(ExitStack())  # closed before the combine phase
        rp = mid.enter_context(tc.tile_pool(name="route", bufs=1))
        rps = mid.enter_context(tc.tile_pool(name="routes", bufs=2))
        with ExitStack() as ph:
            psr = ph.enter_context(tc.tile_pool(name="psr", bufs=2, space="PSUM"))

            w_sb = rp.tile([P, BLK * E], F32, tag="wsb")
            for s in range(N_CORES):
                src = pay_all[s * (NSH + 1): s * (NSH + 1) + NSH, :]
                nc.sync.dma_start(
                    out=w_sb[s * SPP:(s + 1) * SPP, :],
                    in_=src.rearrange("(pp bb) e -> pp (bb e)", pp=SPP))

            ssall = rp.tile([N_CORES, E], F32, tag="ssall")
            for s in range(N_CORES):
                r0 = s * (NSH + 1) + NSH
                nc.sync.dma_start(out=ssall[s:s + 1, :], in_=pay_all[r0:r0 + 1, :])

            ch_e, rank_e, vl_e = [], [], []
            cnt_cols = rp.tile([BLK, E], F32, tag="cntcols")
            for e in range(E):
                w_e = w_sb[:].rearrange("p (b e) -> p b e", e=E)[:, :, e]
                ch = rp.tile([P, BLK], F32, tag=f"ch{e}")
                nc.vector.tensor_scalar(ch[:], w_e, 0.0, None, op0=OP.is_gt)
                incl = rps.tile([P, BLK], F32, tag="incl")
                nc.vector.tensor_tensor_scan(incl[:], ch[:], zerosB[:], 0.0,
                                             op0=OP.add, op1=OP.add)
                segp = psr.tile([P, 1], F32, tag="segp", bufs=2)
                nc.tensor.matmul(segp[:], lhsT=sut[:], rhs=incl[:, BLK - 1:BLK],
                                 start=True, stop=True)
                rank = rp.tile([P, BLK], F32, tag=f"rank{e}")
                nc.vector.tensor_tensor(out=rank[:], in0=incl[:], in1=ch[:],
                                        op=OP.subtract)
                segps = rps.tile([P, 1], F32, tag="segps")
                nc.vector.tensor_copy(segps[:], segp[:])
                nc.vector.tensor_scalar_add(rank[:], rank[:], segps[:])
                vl = rp.tile([P, BLK], F32, tag=f"vl{e}")
                nc.vector.tensor_scalar(vl[:], rank[:], float(cap), None, op0=OP.is_lt)
                nc.vector.tensor_tensor(out=vl[:], in0=vl[:], in1=ch[:], op=OP.mult)
                # per-b column counts for aux: cnt_cols[:, e] = sum_p ch[p, :]
                ccol = psr.tile([BLK, 1], F32, tag="ccol", bufs=2)
                nc.tensor.matmul(ccol[:], lhsT=ch[:], rhs=ones[:], start=True,
                                 stop=True)
                nc.vector.tensor_copy(cnt_cols[:, e:e + 1], ccol[:])
                ch_e.append(ch); rank_e.append(rank); vl_e.append(vl)

            # ---- aux loss ----
            cnt_ps = psr.tile([E, 1], F32, tag="cntps", bufs=1)
            nc.tensor.matmul(cnt_ps[:], lhsT=cnt_cols[:], rhs=ones[0:BLK, :],
                             start=True, stop=True)
            sstot_ps = psr.tile([E, 1], F32, tag="sstot", bufs=1)
            nc.tensor.matmul(sstot_ps[:], lhsT=ssall[:], rhs=ones[0:N_CORES, :],
                             start=True, stop=True)
            cnt_sb = rps.tile([E, 1], F32, tag="cntsb")
            nc.vector.tensor_copy(cnt_sb[:], cnt_ps[:])
            prod = rps.tile([E, 1], F32, tag="prod")
            nc.vector.tensor_tensor(out=prod[:], in0=cnt_sb[:], in1=sstot_ps[:],
                                    op=OP.mult)
            aux_ps = psr.tile([1, 1], F32, tag="auxps", bufs=1)
            nc.tensor.matmul(aux_ps[:], lhsT=prod[:], rhs=ones[0:E, :],
                             start=True, stop=True)
            aux_sb = rps.tile([1, 1], F32, tag="auxsb")
            scale = cfg.aux_coef * cfg.E / (float(N) * float(N))
            nc.scalar.activation(aux_sb[:], aux_ps[:], AF.Copy, scale=scale)
            nc.sync.dma_start(out=aux_out[:], in_=aux_sb[:])

            # ---- combine metadata for all tokens ----
            cnt = rps.tile([P, BLK], F32, tag="cnt")
            nc.vector.memset(cnt[:], 0.0)
            slot1 = rp.tile([P, BLK], F32, tag="slot1")
            slot2 = rp.tile([P, BLK], F32, tag="slot2")
            wm1 = rp.tile([P, BLK], F32, tag="wm1")
            wm2 = rp.tile([P, BLK], F32, tag="wm2")
            for t_ in (slot1, slot2, wm1, wm2):
                nc.vector.memset(t_[:], 0.0)
            for e in range(E):
                w_e = w_sb[:].rearrange("p (b e) -> p b e", e=E)[:, :, e]
                nc.vector.tensor_tensor(out=cnt[:], in0=cnt[:], in1=ch_e[e][:],
                                        op=OP.add)
                sel1 = rps.tile([P, BLK], F32, tag="sel1")
                nc.vector.tensor_scalar(sel1[:], cnt[:], 1.0, None, op0=OP.is_equal)
                nc.vector.tensor_tensor(out=sel1[:], in0=sel1[:], in1=ch_e[e][:],
                                        op=OP.mult)
                sel2 = rps.tile([P, BLK], F32, tag="sel2")
                nc.vector.tensor_scalar(sel2[:], cnt[:], 2.0, None, op0=OP.is_equal)
                nc.vector.tensor_tensor(out=sel2[:], in0=sel2[:], in1=ch_e[e][:],
                                        op=OP.mult)
                slotv = rps.tile([P, BLK], F32, tag="slotv")
                nc.vector.tensor_scalar_min(slotv[:], rank_e[e][:], float(cap))
                nc.vector.tensor_scalar_add(slotv[:], slotv[:], float(e * SLOTS))
                wmv = rps.tile([P, BLK], F32, tag="wmv")
                nc.vector.tensor_tensor(out=wmv[:], in0=w_e, in1=vl_e[e][:],
                                        op=OP.mult)
                tmp = rps.tile([P, BLK], F32, tag="tmpm")
                nc.vector.tensor_tensor(out=tmp[:], in0=sel1[:], in1=slotv[:], op=OP.mult)
                nc.vector.tensor_tensor(out=slot1[:], in0=slot1[:], in1=tmp[:], op=OP.add)
                nc.vector.tensor_tensor(out=tmp[:], in0=sel2[:], in1=slotv[:], op=OP.mult)
                nc.vector.tensor_tensor(out=slot2[:], in0=slot2[:], in1=tmp[:], op=OP.add)
                nc.vector.tensor_tensor(out=tmp[:], in0=sel1[:], in1=wmv[:], op=OP.mult)
                nc.vector.tensor_tensor(out=wm1[:], in0=wm1[:], in1=tmp[:], op=OP.add)
                nc.vector.tensor_tensor(out=tmp[:], in0=sel2[:], in1=wmv[:], op=OP.mult)
                nc.vector.tensor_tensor(out=wm2[:], in0=wm2[:], in1=tmp[:], op=OP.add)

            meta_sb = rp.tile([P, BLK * 4], F32, tag="metasb")
            mview = meta_sb[:].rearrange("p (b k) -> p k b", k=4)
            nc.vector.tensor_copy(mview[:, 0, :], slot1[:])
            nc.vector.tensor_copy(mview[:, 1, :], wm1[:])
            nc.vector.tensor_copy(mview[:, 2, :], slot2[:])
            nc.vector.tensor_copy(mview[:, 3, :], wm2[:])
            nc.sync.dma_start(
                out=meta[:].rearrange("(p b) k -> p (b k)", p=P), in_=meta_sb[:])

            # ---- own-expert dispatch offsets ----
            off = rp.tile([P, BLK], F32, tag="off")
            nc.vector.memset(off[:], 0.0)
            for e in range(E):
                offe = rps.tile([P, BLK], F32, tag="offe")
                nc.vector.tensor_tensor(out=offe[:], in0=rank_e[e][:], in1=vl_e[e][:],
                                        op=OP.mult)
                t2 = rps.tile([P, BLK], F32, tag="offt2")
                nc.vector.tensor_scalar(t2[:], vl_e[e][:], BIG, None, op0=OP.mult)
                nc.vector.tensor_tensor(out=offe[:], in0=offe[:], in1=t2[:],
                                        op=OP.subtract)
                nc.vector.tensor_scalar_add(offe[:], offe[:], BIG)
                nc.vector.tensor_scalar_mul(offe[:], offe[:], esel_sb[:, e:e + 1])
                nc.vector.tensor_tensor(out=off[:], in0=off[:], in1=offe[:], op=OP.add)

            pb_pay = rp.tile([P, BLK * 2], FD, tag="pbpay")
            pview = pb_pay[:].rearrange("p (b k) -> p k b", k=2)
            nc.vector.tensor_copy(pview[:, 0, :], iota_p[:].to_broadcast([P, BLK]))
            nc.vector.tensor_copy(pview[:, 1, :], iota_b[:])

        # =========== Phase C: compaction + gather + FFN ===========
        dp = mid.enter_context(tc.tile_pool(name="disp", bufs=1))
        dps = mid.enter_context(tc.tile_pool(name="disps", bufs=2))
        eqp = mid.enter_context(tc.tile_pool(name="eqs", bufs=2))
        ps_cp = mid.enter_context(tc.tile_pool(name="pscp", bufs=2, space="PSUM"))
        ps_tr = mid.enter_context(tc.tile_pool(name="pstr", bufs=2, space="PSUM"))

        # slot -> token-id table, built chunk by chunk
        idx_tiles = [dp.tile([P, 1], I32, tag=f"idx{g}", name=f"idx{g}") for g in range(NG)]
        idxf_tiles = [dp.tile([P, 1], F32, tag=f"idxf{g}", name=f"idxf{g}") for g in range(NG)]
        for g in range(NG):
            nc.vector.memset(idxf_tiles[g][:], 0.0)

        cp_sbs = {}
        for (s0, w) in cfg.eq_chunks():
            cp_ps = ps_cp.tile([2, w], F32, tag="cpps", bufs=1)
            for b in range(BLK):
                eq = eqp.tile([P, w], FD, tag="eq")
                nc.vector.tensor_scalar(eq[:], eq_iotas[s0][:], off[:, b:b + 1],
                                        None, op0=OP.is_equal)
                nc.tensor.matmul(cp_ps[:], lhsT=pb_pay[:, 2 * b:2 * b + 2],
                                 rhs=eq[:],
                                 start=(b == 0), stop=(b == BLK - 1))
            cp_sb = dp.tile([2, w], F32, tag=f"cpsb{s0}")
            nc.vector.tensor_copy(cp_sb[:], cp_ps[:])
            cp_sbs[s0] = cp_sb

        # reconstruct token ids per 128-slot group: id = p*BLK + b
        for (s0, w) in cfg.eq_chunks():
            cp_sb = cp_sbs[s0]
            pos = 0
            while pos < w:
                gslot = s0 + pos
                g = gslot // P
                goff = gslot - g * P
                pw = min(P - goff, w - pos)
                trp = ps_tr.tile([P, 2], F32, tag="trp", bufs=1)
                nc.tensor.transpose(trp[0:pw, :], cp_sb[:, pos:pos + pw],
                                    ident[0:2, 0:2])
                trs = dps.tile([P, 2], F32, tag="trs")
                nc.vector.tensor_copy(trs[0:pw, :], trp[0:pw, :])
                nc.vector.tensor_scalar(idxf_tiles[g][goff:goff + pw, :],
                                        trs[0:pw, 0:1], float(BLK), trs[0:pw, 1:2],
                                        op0=OP.mult, op1=OP.add)
                pos += pw
        for g in range(NG):
            nc.vector.tensor_copy(idx_tiles[g][:], idxf_tiles[g][:])

        # FFN over slot chunks, h-chunks processed in blocks of HB so that
        # h tiles and w2 tiles stay small; mm2 accumulates across blocks in SBUF.
        HB = min(4, NH)
        assert NH % HB == 0
        wp1 = mid.enter_context(tc.tile_pool(name="w1p", bufs=3 * ND))
        wp2 = mid.enter_context(tc.tile_pool(name="w2p", bufs=1))
        hp = mid.enter_context(tc.tile_pool(name="hp", bufs=2))
        yp = mid.enter_context(tc.tile_pool(name="yp", bufs=1))
        xgp = mid.enter_context(tc.tile_pool(name="xgp", bufs=3))
        ps_mm1 = mid.enter_context(tc.tile_pool(name="psmm1", bufs=2, space="PSUM"))
        ps_mm2 = mid.enter_context(tc.tile_pool(name="psmm2", bufs=2, space="PSUM"))

        chunks = cfg.ffn_chunks()
        dsub = [(c0, min(512, D - c0)) for c0 in range(0, D, 512)]

        for (c0, cw) in chunks:
            # gather + transpose x rows for this chunk's slots
            xT_c = [dp.tile([P, cw], FD, tag=f"xT{dc}", name=f"xTc{c0}_{dc}",
                            padded_shape=[P, max(w for _, w in chunks)])
                    for dc in range(ND)]
            for gl in range(cw // P):
                g = c0 // P + gl
                xg = xgp.tile([P, D], F32, tag="xg")
                nc.gpsimd.indirect_dma_start(
                    out=xg[:], out_offset=None, in_=x_full[:],
                    in_offset=bass.IndirectOffsetOnAxis(ap=idx_tiles[g][:, :1], axis=0))
                for dc in range(ND):
                    xtp = ps_tr.tile([P, P], F32, tag="xtp")
                    nc.tensor.transpose(xtp[:], xg[:, dc * P:(dc + 1) * P], ident[:])
                    nc.vector.tensor_copy(xT_c[dc][:, gl * P:(gl + 1) * P], xtp[:])

            ntt = cw // P
            y_accs = {}
            for hbi, hb in enumerate(range(0, NH, HB)):
                # mm1 + silu for this h block: h = silu(w1t.T @ xT), [h, tok]
                h_blk = []
                for hl in range(HB):
                    hc = hb + hl
                    w1_tiles_d = []
                    for dc in range(ND):
                        wt = wp1.tile([P, P], FD, tag="w1t", name="w1tl")
                        nc.gpsimd.dma_start(
                            out=wt[:],
                            in_=w1t[dc * P:(dc + 1) * P, hc * P:(hc + 1) * P])
                        w1_tiles_d.append(wt)
                    ht = hp.tile([P, cw], FD, tag=f"h{hl}", name=f"h{hl}",
                                 padded_shape=[P, max(w for _, w in chunks)])
                    for (s0, sw) in cfg.mm_subs(cw):
                        ps1 = ps_mm1.tile([P, sw], F32, tag="ps1")
                        for dc in range(ND):
                            nc.tensor.matmul(ps1[:], lhsT=w1_tiles_d[dc][:],
                                             rhs=xT_c[dc][:, s0:s0 + sw],
                                             start=(dc == 0), stop=(dc == ND - 1))
                        sg = wp1.tile([P, sw], F32, tag="sg", bufs=2, name="sg")
                        nc.scalar.activation(sg[:], ps1[:], AF.Sigmoid)
                        nc.vector.tensor_tensor(out=ht[:, s0:s0 + sw],
                                                in0=sg[:], in1=ps1[:], op=OP.mult)
                    h_blk.append(ht)

                # partial mm2 for this h block into SBUF accumulators
                for (d0, dw) in dsub:
                    w2_blk = []
                    for hl in range(HB):
                        hc = hb + hl
                        w2_t = wp2.tile([P, dw], FD, tag="w2t", bufs=2 * HB,
                                        name="w2tl")
                        nc.gpsimd.dma_start(
                            out=w2_t[:], in_=w2t[hc * P:(hc + 1) * P, d0:d0 + dw])
                        w2_blk.append(w2_t)
                    for tt in range(ntt):
                        ps2 = ps_mm2.tile([P, dw], F32, tag="ps2")
                        for hl in range(HB):
                            nc.tensor.matmul(
                                ps2[:], lhsT=h_blk[hl][:, tt * P:(tt + 1) * P],
                                rhs=w2_blk[hl][:],
                                start=(hl == 0), stop=(hl == HB - 1))
                        if hbi == 0:
                            ya = yp.tile([P, dw], F32, tag=f"ya{tt}_{d0}",
                                         name=f"ya{tt}_{d0}")
                            y_accs[(tt, d0)] = ya
                            nc.vector.tensor_copy(ya[:], ps2[:])
                        else:
                            ya = y_accs[(tt, d0)]
                            nc.vector.tensor_tensor(out=ya[:], in0=ya[:],
                                                    in1=ps2[:], op=OP.add)

            for (d0, dw) in dsub:
                for tt in range(ntt):
                    r0 = c0 + tt * P
                    nc.sync.dma_start(out=y_loc[r0:r0 + P, d0:d0 + dw],
                                      in_=y_accs[(tt, d0)][:])

        nc.gpsimd.collective_compute(
            "AllGather", OP.bypass, replica_groups=rg,
            ins=[y_loc[:]], outs=[y_all[:]],
        )

        # =========== Phase D: combine own tokens ===========
        mid.close()
        cbp = top.enter_context(tc.tile_pool(name="comb", bufs=3))
        for rt in range(NRT):
            tid = cbp.tile([P, 1], I32, tag="tid")
            nc.sync.dma_start(out=tid[:], in_=tok_ids[rt * P:(rt + 1) * P, :])
            mg = cbp.tile([P, 4], F32, tag="mg")
            nc.gpsimd.indirect_dma_start(
                out=mg[:], out_offset=None, in_=meta[:],
                in_offset=bass.IndirectOffsetOnAxis(ap=tid[:, :1], axis=0))
            s1 = cbp.tile([P, 1], I32, tag="s1")
            nc.vector.tensor_copy(s1[:], mg[:, 0:1])
            s2 = cbp.tile([P, 1], I32, tag="s2")
            nc.vector.tensor_copy(s2[:], mg[:, 2:3])
            g1 = cbp.tile([P, D], F32, tag="g1")
            nc.gpsimd.indirect_dma_start(
                out=g1[:], out_offset=None, in_=y_all[:],
                in_offset=bass.IndirectOffsetOnAxis(ap=s1[:, :1], axis=0))
            g2 = cbp.tile([P, D], F32, tag="g2")
            nc.gpsimd.indirect_dma_start(
                out=g2[:], out_offset=None, in_=y_all[:],
                in_offset=bass.IndirectOffsetOnAxis(ap=s2[:, :1], axis=0))
            t1 = cbp.tile([P, D], F32, tag="t1")
            nc.vector.tensor_scalar_mul(t1[:], g1[:], mg[:, 1:2])
            ot = cbp.tile([P, D], F32, tag="ot")
            nc.vector.scalar_tensor_tensor(out=ot[:], in0=g2[:], scalar=mg[:, 3:4],
                                           in1=t1[:], op0=OP.mult, op1=OP.add)
            nc.sync.dma_start(out=out_shard[rt * P:(rt + 1) * P, :], in_=ot[:])

    nc.compile()
    return nc


# ---------------- host-side wrapper ----------------

_CACHE = {}


def _get_nc(cfg: Cfg):
    key = (cfg.N, cfg.D, cfg.H, cfg.E, str(cfg.ffn_dtype))
    if key not in _CACHE:
        _CACHE[key] = build_nc(cfg)
    return _CACHE[key]


def make_in_maps(cfg: Cfg, x, gate_w, w1, w2):
    N, D, NSH = cfg.N, cfg.D, cfg.NSH
    xf = np.ascontiguousarray(np.asarray(x, dtype=np.float32).reshape(N, D))
    gate_w = np.asarray(gate_w, dtype=np.float32)
    w1 = np.asarray(w1, dtype=np.float32)
    w2 = np.asarray(w2, dtype=np.float32)
    gwT = np.ascontiguousarray(gate_w.T)
    in_maps = []
    for c in range(N_CORES):
        esel = np.zeros((P, cfg.E), np.float32)
        esel[:, c % cfg.E] = 1.0
        in_maps.append({
            "x_full": xf,
            "xTs": np.ascontiguousarray(xf[c * NSH:(c + 1) * NSH].T),
            "gwT": gwT,
            "w1t": np.ascontiguousarray(w1[c].T),
            "w2t": np.ascontiguousarray(w2[c].T),
            "esel": esel,
            "tok_ids": np.arange(c * NSH, (c + 1) * NSH, dtype=np.int32)[:, None],
        })
    return in_maps


def run_cfg(cfg: Cfg, x, gate_w, w1, w2, trace=False, **kw):
    nc = _get_nc(cfg)
    in_maps = make_in_maps(cfg, x, gate_w, w1, w2)
    res = run_bass_kernel_spmd(nc, in_maps, list(range(N_CORES)), trace=trace, **kw)
    out = np.concatenate(
        [res.results[c]["out_shard"] for c in range(N_CORES)], axis=0)
    out = out.reshape(cfg.B, cfg.T, cfg.D)
    aux = np.float32(res.results[0]["aux_out"].reshape(()))
    return (out, aux), res


def kernel(x, gate_w, w1, w2):
    cfg = Cfg()
    (out, aux), _ = run_cfg(cfg, x, gate_w, w1, w2)
    return out, aux
